# revision 7
# baseline (speedup 1.0000x reference)
"""Contrastive loss kernel for 8 Trainium2 NeuronCores.

Math (reference):
    s = cosine similarity matrix of x [8192, 256]
    d_i = sum_j exp(s_ij * m_ij / tau)   (m zeroes the diagonal -> diag term = 1)
    v_i = s[i, i^1]                      (adjacent-row positive pairs)
    loss = mean(log d_i - v_i / tau)

Distribution: row-shard across 8 cores. Host normalizes rows of x (0.01% of
the FLOPs), transposes to xnT [256, 8192], and per core ROTATES the columns
so each core's own 1024 rows sit at columns 0..1023.  That makes the SPMD
program position-independent: the diagonal/pair blocks are always at a fixed
(compile-time) location, while row sums are invariant to column order.

Device (per core, identical program):
    - big matmul  s_tile = xnT[:, m*128:...].T @ xnT   (bf16 in, fp32 PSUM;
      fp32r measured ~5x slower on HW despite the cost model)
    - fused exp+row-sum on the scalar engine (accum_out), reading PSUM
      supertiles [128, 2048], writing bf16 exp values to SBUF
    - exp(s_ii/tau) and exp(v_i/tau) extracted from the s=0 exp tile with
      mask-multiply-reduce on the vector engine (SBUF only)
Host combines: d_i = rowsum - exp_diag + 1; loss = mean(log d - log exp_v).
Measured ~78.9us/core one-shot (incl. 4MB input DMA) via For_i repeat-diff;
PE-bound (PE-only floor 73us: 256 MMs x [213ns stream + ~53ns LDW]).

NOTE on structure: walrus codegen allows at most ONE semaphore wait per
engine instruction, so the program is arranged so every instruction depends
on at most one not-yet-observed engine (warmup touches + observer copies).
"""

import os
import sys

import numpy as np

sys.path.insert(0, "/opt/trn_rl_repo")

import concourse.bass as bass
import concourse.tile as tile
from concourse import mybir
from concourse.bass_utils import run_bass_kernel_spmd

import os as _os_early
TAU = 0.1
N = 8192
D = 256
P = 128
NCORES = 8
ROWS_PER_CORE = N // NCORES          # 1024
M_TILES = ROWS_PER_CORE // P         # 8
SUPER = int(_os_early.environ.get("KERNEL_SUPER", "2048"))  # ACT supertile width
S_TILES = N // SUPER
SUB = SUPER // 512                   # matmuls of N=512 per supertile
CHUNK = 2048                         # input DMA chunk width
C_TILES = N // CHUNK
FP32 = mybir.dt.float32
FP32R = mybir.dt.float32r
FP8 = mybir.dt.float8e4
# matmul input dtype: "fp32r", "bf16", or "fp8" (fp8e4 + DoubleRow: K=256 in
# one pass at 2 rows/cycle -> half the PE streaming of bf16)
import os as _os
MM_DT = _os.environ.get("KERNEL_MM_DT", "fp8")
EO_DT = _os.environ.get("KERNEL_EO_DT", "bf16")   # exp-output dtype knob
FP8_SCALE = 16.0   # host multiplies xn by this before e4m3 rounding

_CACHE = {}


def build_nc(repeat=1):
    mmdt = {"fp32r": FP32R, "bf16": mybir.dt.bfloat16, "fp8": FP8}[MM_DT]
    xtdt = {"fp32r": FP32, "bf16": mybir.dt.bfloat16, "fp8": FP8}[MM_DT]
    nc = bass.Bass(trn_type="TRN2")
    xt_d = nc.declare_dram_parameter("xt", [2, P, N], xtdt, isOutput=False)
    eodt_d = FP32 if EO_DT == "fp32" else mybir.dt.bfloat16
    eye_d = nc.declare_dram_parameter("eye", [P, P], eodt_d, isOutput=False)
    pm_d = nc.declare_dram_parameter("pm", [P, P], eodt_d, isOutput=False)
    acc_d = nc.declare_dram_parameter("acc", [P, M_TILES * S_TILES], FP32, isOutput=True)
    dv_d = nc.declare_dram_parameter("dv", [P, 2 * M_TILES], FP32, isOutput=True)

    with tile.TileContext(nc) as tc:
        with (
            tc.tile_pool(name="big", bufs=2) as big,
            tc.tile_pool(name="small", bufs=1) as small,
            tc.tile_pool(name="scratch", bufs=4) as sc,
            tc.tile_pool(name="psum", bufs=int(_os.environ.get("KERNEL_PSUM_BUFS", "2")), space="PSUM") as pp,
        ):
            eodt = FP32 if EO_DT == "fp32" else mybir.dt.bfloat16
            eye = small.tile([P, P], eodt, tag="eye")
            pm = small.tile([P, P], eodt, tag="pm")
            acc_sb = small.tile([P, M_TILES * S_TILES], FP32, tag="accsb")
            dv_sb = small.tile([P, 2 * M_TILES], FP32, tag="dvsb")

            nc.sync.dma_start(out=eye, in_=eye_d[:, :])
            nc.sync.dma_start(out=pm, in_=pm_d[:, :])
            # Warmup: make DVE/ACT observe the mask DMAs (and load the Exp
            # table) before the main loop, so steady-state instructions carry
            # a single sem wait (codegen limit) and the ~2.7us ACT table load
            # happens off the critical path.
            warm_v = small.tile([P, 1], FP32, tag="warm_v")
            warm_v2 = small.tile([P, 1], FP32, tag="warm_v2")
            warm_a = small.tile([P, P], FP32, tag="warm_a")
            warm_s = small.tile([P, 1], FP32, tag="warm_s")
            nc.vector.reduce_sum(warm_v, eye, axis=mybir.AxisListType.X)
            nc.vector.reduce_sum(warm_v2, pm, axis=mybir.AxisListType.X)
            nc.scalar.activation(out=warm_a, in_=pm,
                                 func=mybir.ActivationFunctionType.Exp,
                                 scale=1.0, accum_out=warm_s)

            import contextlib
            loop_ctx = (tc.For_i(0, repeat, 1)
                        if repeat > 1 else contextlib.nullcontext())
            with loop_ctx:
                _compute_body(nc, tc, sc, pp, small, big, xt_d, mmdt,
                              eye, pm, acc_sb, dv_sb)

            if _os.environ.get("KERNEL_PE_ONLY", "0") == "1":
                nc.vector.memset(acc_sb, 0.0)
                nc.vector.memset(dv_sb, 0.0)
            nc.sync.dma_start(out=acc_d[:, :], in_=acc_sb)
            nc.sync.dma_start(out=dv_d[:, :], in_=dv_sb)
    _split_multi_waits(nc)
    return nc


def _compute_body(nc, tc, sc, pp, small, big, xt_d, mmdt,
                  eye, pm, acc_sb, dv_sb):
    if _os.environ.get("KERNEL_NULL", "0") == "1":
        nc.vector.memset(acc_sb, 0.0)
        nc.vector.memset(dv_sb, 0.0)
        return
    # x tiles live inside the (bench) loop so input DMA pipelines with the
    # previous iteration's compute; in the one-shot kernel this is just the
    # chunked load.
    fp8 = MM_DT == "fp8"
    if fp8:
        # single 3D tile [P, 2, N]: dim1 = k-tile (d half), so DoubleRow
        # matmuls can take (p, 2, free) APs with a regular stride
        xts = big.tile([P, 2, N], mmdt, tag="xts")
        xt0 = xts[:, 0, :]
        xt1 = xts[:, 1, :]
    else:
        xt0 = big.tile([P, N], mmdt, tag="xt0")  # d = 0..127   (k half 0)
        xt1 = big.tile([P, N], mmdt, tag="xt1")  # d = 128..255 (k half 1)
    headopt = _os.environ.get("KERNEL_HEADOPT", "1") == "1"
    if _os.environ.get("KERNEL_CHUNK_DMA", "1") == "1":
        if headopt:
            # split the first chunk pair into 512-wide pieces, k0/k1
            # interleaved, so the first matmul group can start ~1.5us in
            for p_ in range(CHUNK // 512):
                cs = slice(p_ * 512, (p_ + 1) * 512)
                nc.sync.dma_start(out=xt0[:, cs], in_=xt_d[0, :, cs].bitcast(mmdt))
                nc.sync.dma_start(out=xt1[:, cs], in_=xt_d[1, :, cs].bitcast(mmdt))
            first_c = 1
        else:
            first_c = 0
        for c_ in range(first_c, C_TILES):
            cs = slice(c_ * CHUNK, (c_ + 1) * CHUNK)
            nc.sync.dma_start(out=xt0[:, cs], in_=xt_d[0, :, cs].bitcast(mmdt))
            nc.sync.dma_start(out=xt1[:, cs], in_=xt_d[1, :, cs].bitcast(mmdt))
    else:
        nc.sync.dma_start(out=xt0, in_=xt_d[0].bitcast(mmdt))
        nc.sync.dma_start(out=xt1, in_=xt_d[1].bitcast(mmdt))
    if MM_DT in ("bf16", "fp8") and _os.environ.get("KERNEL_HEADOPT", "1") == "1":
        ps_warm = pp.tile([P, SUPER], FP32, tag="super")
        for _w in range(12):
            nc.tensor.matmul(ps_warm[:, 0:P], eye, eye, start=True, stop=True)
    for s in range(S_TILES):
        for m in range(M_TILES):
            if (m == 0 and MM_DT in ("bf16", "fp8") and (s * SUPER) % CHUNK == 0
                    and _os.environ.get("KERNEL_CHUNK_DMA", "1") == "1"):
                # dummy weight loads absorb the chunk-DMA waits on PE
                nc.tensor.ldweights(xt0[:, s * SUPER:s * SUPER + P])
                nc.tensor.ldweights(xt1[:, s * SUPER:s * SUPER + P])
            ps = pp.tile([P, SUPER], FP32, tag="super")
            if fp8:
                lhs = xts[:, 0:2, m * P:(m + 1) * P]
                mmw = int(_os.environ.get("KERNEL_MMW", "512"))
                for sub in range(SUPER // mmw):
                    cols = slice(s * SUPER + sub * mmw, s * SUPER + (sub + 1) * mmw)
                    nc.tensor.matmul(ps[:, sub * mmw:(sub + 1) * mmw],
                                     lhs, xts[:, 0:2, cols],
                                     start=True, stop=True,
                                     perf_mode=mybir.MatmulPerfMode.DoubleRow)
            elif _os.environ.get("KERNEL_K_OUTER", "0") == "1":
                lhs0 = xt0[:, m * P:(m + 1) * P]
                lhs1 = xt1[:, m * P:(m + 1) * P]
                # one weight load serves 4 column slices
                for k, (lhs, xt) in enumerate(((lhs0, xt0), (lhs1, xt1))):
                    for sub in range(SUB):
                        cols = slice(s * SUPER + sub * 512,
                                     s * SUPER + (sub + 1) * 512)
                        nc.tensor.matmul(ps[:, sub * 512:(sub + 1) * 512],
                                         lhs, xt[:, cols],
                                         start=(k == 0), stop=(k == 1))
            else:
                lhs0 = xt0[:, m * P:(m + 1) * P]
                lhs1 = xt1[:, m * P:(m + 1) * P]
                for sub in range(SUB):
                    cols = slice(s * SUPER + sub * 512, s * SUPER + (sub + 1) * 512)
                    pslice = ps[:, sub * 512:(sub + 1) * 512]
                    nc.tensor.matmul(pslice, lhs0, xt0[:, cols],
                                     start=True, stop=False)
                    nc.tensor.matmul(pslice, lhs1, xt1[:, cols],
                                     start=False, stop=True)
            if _os.environ.get("KERNEL_PE_ONLY", "0") == "1":
                continue
            # exp + fused row-sum; s=0 exp tiles keep their own slots
            # because DVE reads them (diag/pair extraction).
            eo = sc.tile([P, SUPER], FP32 if EO_DT == "fp32" else mybir.dt.bfloat16,
                         tag="expout0" if s == 0 else "expout")
            nc.scalar.activation(
                out=eo, in_=ps, func=mybir.ActivationFunctionType.Exp,
                scale=1.0 / ((FP8_SCALE * FP8_SCALE) if fp8 else 1.0) / TAU,
                accum_out=acc_sb[:, m * S_TILES + s:m * S_TILES + s + 1])
            if s == 0:
                gblk = eo[:, m * P:(m + 1) * P]
                tmp = sc.tile([P, P], FP32, tag="gtmp")
                nc.vector.tensor_tensor(
                    out=tmp, in0=gblk, in1=eye, op=mybir.AluOpType.mult)
                nc.vector.reduce_sum(
                    dv_sb[:, m:m + 1], tmp, axis=mybir.AxisListType.X)
                tmp2 = sc.tile([P, P], FP32, tag="gtmp")
                nc.vector.tensor_tensor(
                    out=tmp2, in0=gblk, in1=pm, op=mybir.AluOpType.mult)
                nc.vector.reduce_sum(
                    dv_sb[:, M_TILES + m:M_TILES + m + 1], tmp2,
                    axis=mybir.AxisListType.X)
                # observer: let ACT see the DVE sem so the next
                # s=0 exp's buffer WAR needs no extra wait
                obs = small.tile([P, 1], FP32, tag=f"obs{m}")
                nc.scalar.copy(out=obs,
                               in_=dv_sb[:, M_TILES + m:M_TILES + m + 1])


def _split_multi_waits(nc):
    """walrus codegen accepts at most ONE semaphore wait per engine
    instruction; Tile's wait assignment can bake in several.  Hoist all but
    the last wait of each engine instruction into standalone
    InstEventSemaphore sequencer ops right before it (the same mechanism
    barriers use) — semantics are identical, the engine blocks on the waits
    in order."""
    n_split = 0
    for blk in nc.m.functions[0].blocks:
        new_insts = []
        for inst in blk.instructions:
            si = inst.sync_info
            tname = type(inst).__name__
            if si is not None and len(si.on_wait) > 1 and tname != "InstEventSemaphore":
                waits = list(si.on_wait)
                for j, w in enumerate(waits[:-1]):
                    es = mybir.InstEventSemaphore(
                        name=f"W-split-{inst.name}-{j}")
                    es.engine = inst.engine
                    es.sync_info = mybir.SyncInfo(on_wait=[w], on_update=[])
                    new_insts.append(es)
                    nc.register_instruction(es)
                    n_split += 1
                inst.sync_info = mybir.SyncInfo(
                    on_wait=[waits[-1]], on_update=list(si.on_update))
            new_insts.append(inst)
        blk.instructions[:] = new_insts
    return n_split


def _masks():
    if EO_DT == "fp32":
        mdt = np.float32
    else:
        import ml_dtypes
        mdt = ml_dtypes.bfloat16
    eye = np.eye(P, dtype=mdt)
    pm = np.zeros((P, P), dtype=mdt)
    idx = np.arange(P)
    pm[idx, idx ^ 1] = mdt(1.0)
    return eye, pm


def _prepare_inputs(x):
    x = np.ascontiguousarray(np.asarray(x, dtype=np.float32))
    inv = 1.0 / np.sqrt((x * x).sum(axis=1))
    xn = x * inv[:, None].astype(np.float32)
    if MM_DT == "fp8":
        import ml_dtypes
        # scale up before e4m3 rounding so typical entries (~0.06) land in
        # the normal range; the ACT exp scale divides the product back out
        xnT = np.ascontiguousarray(
            (xn.T * FP8_SCALE).astype(ml_dtypes.float8_e4m3))
    elif MM_DT == "bf16":
        import ml_dtypes
        xnT = np.ascontiguousarray(xn.T.astype(ml_dtypes.bfloat16))
    else:
        xnT = np.ascontiguousarray(xn.T.astype(np.float32))  # [256, 8192]
    eye, pm = _masks()
    in_maps = []
    for c in range(NCORES):
        rolled = np.roll(xnT, -c * ROWS_PER_CORE, axis=1)
        xt = np.ascontiguousarray(rolled.reshape(2, P, N))
        in_maps.append({"xt": xt, "eye": eye, "pm": pm})
    return in_maps


def _combine(results):
    total = 0.0
    for c in range(NCORES):
        acc = np.asarray(results[c]["acc"], dtype=np.float64)   # [128, 32]
        dv = np.asarray(results[c]["dv"], dtype=np.float64)     # [128, 16]
        rowsum = acc.reshape(P, M_TILES, S_TILES).sum(axis=2)   # [p, m]
        diag_exp = dv[:, :M_TILES]                              # exp(s_ii/tau)
        v_exp = dv[:, M_TILES:]                                 # exp(v_i/tau)
        d = rowsum - diag_exp + 1.0
        total += (np.log(d) - np.log(v_exp)).sum()
    return np.float32(total / N)


def kernel(x, repeat=None):
    if repeat is None:
        repeat = int(os.environ.get("KERNEL_REPEAT", "1"))
    key = f"nc{repeat}"
    if key not in _CACHE:
        _CACHE[key] = build_nc(repeat)
    nc = _CACHE[key]
    in_maps = _prepare_inputs(x)
    trace = bool(int(os.environ.get("KERNEL_TRACE", "0")))
    res = run_bass_kernel_spmd(nc, in_maps, list(range(NCORES)), trace=trace)
    _CACHE["last_results"] = res
    return _combine(res.results)



# revision 8
# speedup vs baseline: 1.0355x; 1.0355x over previous
"""Contrastive loss v2: symmetric (upper-triangle) computation on 8 cores.

Math (reference):
    s = cosine similarity matrix of x [8192, 256] (symmetric!)
    d_i = sum_j exp(s_ij/tau) with diag term replaced by 1
    v_i = s[i, i^1];  loss = mean(log d_i - v_i/tau)

v1 computed the FULL s row-block per core (every entry twice globally).
v2 exploits symmetry: each unordered 512-row-group block pair {gi, gj} is
computed ONCE; its exp supplies BOTH row sums (ACT accum / DVE reduce for
rows of gi) and column sums (PE ones-matmul -> [1, W] PSUM strips, DMA'd
out; host scatters to rows of gj).

SPMD via the circulant/roll trick: core c's input columns are rolled by
-c*1024, so one fixed program covers, across cores, all pairs with group
differences delta = 1..8 from both parities. In rolled coords core c
computes two row strips:
    r=0: rows [0:512)    x cols [m*128 : 4608)      (per m-subtile)
    r=1: rows [512:1024) x cols [512 + m*128 : 5120)
Each strip = diag upper-triangle + delta=1..7 once each + delta=8 (which is
covered twice globally; the host drops the duplicate from cores 4-7).

Engines: PE fp8e4+DoubleRow matmuls (K=256 in one 512-wide pass) + bf16
ones-matmul column sums; ACT exp (no accum -> 3 big chunks per (r,m));
DVE row-sum reductions (bf16 2x) + diag/pair mask extraction.
"""

import os
import sys

import numpy as np

sys.path.insert(0, "/opt/trn_rl_repo")

import concourse.bass as bass
import concourse.tile as tile
from concourse import mybir
from concourse.bass_utils import run_bass_kernel_spmd

TAU = 0.1
N = 8192
D = 256
P = 128
NCORES = 8
FP8_SCALE = 16.0
XCOLS = 5120          # rolled cols a core touches: [0, 512 + 4608)
SW = 4608             # strip width in u-coords (relative to 512*r)
CW = 1536             # ACT/PSUM supertile chunk width (3 chunks per strip)
NCH = SW // CW        # 3
FP32 = mybir.dt.float32
FP8 = mybir.dt.float8e4
BF16 = mybir.dt.bfloat16

# rowsum segments per (r, m) = the three ACT/PSUM chunks. The delta=8 dup
# range [4096, 4608) is NOT split out: for cores 4-7 the host subtracts the
# partner core's k=8 column-sum strip instead (bit-identical values).
RS_SEGS = [(0, 1536), (1536, 3072), (3072, 4608)]  # seg0 start is max(128m, 0)

_CACHE = {}


def _quantize(x):
    """normalize rows, scale, quantize to e4m3; returns [256, 8192] fp8."""
    import ml_dtypes
    x = np.ascontiguousarray(np.asarray(x, dtype=np.float32))
    inv = (1.0 / np.sqrt((x * x).sum(axis=1))).astype(np.float32)
    xn = x * inv[:, None]
    return np.ascontiguousarray((xn.T * FP8_SCALE).astype(ml_dtypes.float8_e4m3))


def _prepare_inputs(x):
    xq = _quantize(x)
    eye, pm = _masks()
    in_maps = []
    for c in range(NCORES):
        rolled = np.roll(xq, -c * 2 * 512, axis=1)[:, :XCOLS]
        # [P, 2, XCOLS]: partition-major so one DMA moves both k-halves
        xt = np.ascontiguousarray(
            rolled.reshape(2, P, XCOLS).transpose(1, 0, 2))
        in_maps.append({"xt": xt, "eye": eye, "pm": pm,
                        "ones1": np.ones((P, 1), dtype=_bf16())})
    return in_maps


def _bf16():
    import ml_dtypes
    return ml_dtypes.bfloat16


def _masks():
    mdt = _bf16()
    eye = np.eye(P, dtype=mdt)
    pm = np.zeros((P, P), dtype=mdt)
    idx = np.arange(P)
    pm[idx, idx ^ 1] = mdt(1.0)
    return eye, pm


def mock_core_outputs(x, c):
    """Numpy model of what core c's device program produces (exact fp32
    math, no quantization) — for validating the indexing/combine."""
    x = np.asarray(x, dtype=np.float32)
    xn = x / np.sqrt((x * x).sum(axis=1, keepdims=True))
    xnT = xn.T                                   # [256, 8192]
    rolled = np.roll(xnT, -c * 1024, axis=1)[:, :XCOLS]   # [256, 5120]
    acc = np.zeros((P, 32), dtype=np.float64)
    dv = np.zeros((P, 16), dtype=np.float64)
    cs = np.zeros((2, 9, 512), dtype=np.float64)
    for r in range(2):
        br = 512 * r
        rows = rolled[:, br:br + 512]            # [256, 512] own rows
        # s strip: [512, 4608]
        s = rows.T @ rolled[:, br:br + SW]       # [512 rows, 4608 u-cols]
        e = np.exp(s / TAU)
        for m in range(4):
            em = e[m * 128:(m + 1) * 128]        # [128, 4608]
            for j, (u0, u1) in enumerate(RS_SEGS):
                u0 = max(u0, 128 * m)
                acc[:, (r * 4 + m) * 3 + j] = em[:, u0:u1].sum(axis=1)
            dblk = em[:, 128 * m:128 * (m + 1)]
            dv[:, r * 4 + m] = np.diag(dblk)
            idx = np.arange(P)
            dv[:, 8 + r * 4 + m] = dblk[idx, idx ^ 1]
        # column sums
        for k in range(1, 9):
            u0 = 512 * k
            cs[r, k, :] = e[:, u0:u0 + 512].sum(axis=0)
        # diag-triangle columns [128, 512): only rows strictly above
        for mp in range(3):
            u0, u1 = 128 * (mp + 1), 512
            cs[r, 0, u0:u1] += e[128 * mp:128 * (mp + 1), u0:u1].sum(axis=0)
    return {"acc": acc, "dv": dv, "cs": cs}


def _combine(results):
    d = np.zeros(N, dtype=np.float64)
    diag_exp = np.zeros(N, dtype=np.float64)
    v_exp = np.zeros(N, dtype=np.float64)
    for c in range(NCORES):
        acc = np.asarray(results[c]["acc"], dtype=np.float64)
        dv = np.asarray(results[c]["dv"], dtype=np.float64)
        cs = np.asarray(results[c]["cs"], dtype=np.float64)
        for r in range(2):
            g = (2 * c + r) % 16
            rowbase = g * 512
            for m in range(4):
                rows = rowbase + m * 128 + np.arange(P)
                b = (r * 4 + m) * 3
                d[rows] += acc[:, b:b + 3].sum(axis=1)
                diag_exp[rows] = dv[:, r * 4 + m]
                v_exp[rows] = dv[:, 8 + r * 4 + m]
            if c >= 4:
                # delta=8 block was also computed (transposed) by core c-4;
                # subtract the duplicate via the partner's k=8 colsum strip,
                # whose values are bit-identical to our fused rowsum part
                partner = np.asarray(results[c - 4]["cs"], dtype=np.float64)
                d[rowbase:rowbase + 512] -= partner[r, 8, :]
            for k in range(9):
                if k == 8 and c >= 4:
                    continue                     # delta=8 colsum: add once
                if k == 0:
                    u = np.arange(128, 512)
                    vals = cs[r, 0, 128:512]
                else:
                    u = np.arange(512 * k, 512 * (k + 1))
                    vals = cs[r, k, :]
                cols = (512 * r + u + 1024 * c) % N
                d[cols] += vals
    dfin = d - diag_exp + 1.0
    total = (np.log(dfin) - np.log(v_exp)).sum()
    return np.float32(total / N)


def mock_kernel(x):
    return _combine([mock_core_outputs(x, c) for c in range(NCORES)])


def _split_multi_waits(nc):
    """walrus codegen accepts at most ONE semaphore wait per engine
    instruction; hoist extra waits into standalone InstEventSemaphore ops."""
    n_split = 0
    for blk in nc.m.functions[0].blocks:
        new_insts = []
        for inst in blk.instructions:
            si = inst.sync_info
            tname = type(inst).__name__
            if si is not None and len(si.on_wait) > 1 and tname != "InstEventSemaphore":
                waits = list(si.on_wait)
                for j, w in enumerate(waits[:-1]):
                    es = mybir.InstEventSemaphore(
                        name=f"W-split-{inst.name}-{j}")
                    es.engine = inst.engine
                    es.sync_info = mybir.SyncInfo(on_wait=[w], on_update=[])
                    new_insts.append(es)
                    nc.register_instruction(es)
                    n_split += 1
                inst.sync_info = mybir.SyncInfo(
                    on_wait=[waits[-1]], on_update=list(si.on_update))
            new_insts.append(inst)
        blk.instructions[:] = new_insts
    return n_split


DR = mybir.MatmulPerfMode.DoubleRow
EXP_SCALE = 1.0 / (FP8_SCALE * FP8_SCALE * TAU)


def build_nc(repeat=1):
    import os as _os
    nc = bass.Bass(trn_type="TRN2")
    xt_d = nc.declare_dram_parameter("xt", [P, 2, XCOLS], FP8, isOutput=False)
    eye_d = nc.declare_dram_parameter("eye", [P, P], BF16, isOutput=False)
    pm_d = nc.declare_dram_parameter("pm", [P, P], BF16, isOutput=False)
    ones_d = nc.declare_dram_parameter("ones1", [P, 1], BF16, isOutput=False)
    out_d = nc.declare_dram_parameter("accdv", [P, 40], FP32, isOutput=True)
    cs_d = nc.declare_dram_parameter("cs", [2, 9, 512], FP32, isOutput=True)

    with tile.TileContext(nc) as tc:
        with (
            tc.tile_pool(name="big", bufs=2) as big,
            tc.tile_pool(name="small", bufs=1) as small,
            tc.tile_pool(name="eop", bufs=3) as eop,
            tc.tile_pool(name="sc", bufs=4) as scp,
            tc.tile_pool(name="psum", bufs=2, space="PSUM") as pp,
        ):
            eye = small.tile([P, P], BF16, tag="eye")
            pm = small.tile([P, P], BF16, tag="pm")
            ones1 = small.tile([P, 1], BF16, tag="ones1")
            outsb = small.tile([P, 40], FP32, tag="outsb")
            acc_sb = outsb[:, 0:24]
            dv_sb = outsb[:, 24:40]

            # masks go through the software DGE (gpsimd) so the HWDGE queue
            # is free for the input chunks; warmups run off a memset tile so
            # neither the Exp-table load nor the PE pstate ramp waits on DMA
            nc.gpsimd.dma_start(out=eye, in_=eye_d[:, :])
            nc.gpsimd.dma_start(out=pm, in_=pm_d[:, :])
            nc.gpsimd.dma_start(out=ones1, in_=ones_d[:, :])
            warm_in = small.tile([P, P], BF16, tag="warm_in")
            warm_a = small.tile([P, P], FP32, tag="warm_a")
            warm_s = small.tile([P, 1], FP32, tag="warm_s")
            nc.vector.memset(warm_in, 0.0)
            nc.scalar.activation(out=warm_a, in_=warm_in,
                                 func=mybir.ActivationFunctionType.Exp,
                                 scale=1.0, accum_out=warm_s)
            ps_warm = pp.tile([P, 512], FP32, tag="csp")
            for _w in range(12):
                nc.tensor.matmul(ps_warm[:, 0:P], warm_in, warm_in,
                                 start=True, stop=True)

            import contextlib
            loop_ctx = (tc.For_i(0, repeat, 1)
                        if repeat > 1 else contextlib.nullcontext())
            with loop_ctx:
                _compute_body(nc, tc, pp, eop, scp, big, small,
                              xt_d, cs_d, eye, pm, ones1, acc_sb, dv_sb)

            nc.sync.dma_start(out=out_d[:, :], in_=outsb)
    _split_multi_waits(nc)
    return nc


def _chunk_pieces(m, c):
    """s-matmul piece u-ranges for (m, chunk c): 512-wide, first piece of
    c0 starts at m*128."""
    u0 = CW * c
    if c == 0:
        return [(128 * m, 512), (512, 1024), (1024, 1536)]
    return [(u0, u0 + 512), (u0 + 512, u0 + 1024), (u0 + 1024, u0 + 1536)]


def _compute_body(nc, tc, pp, eop, scp, big, small,
                  xt_d, cs_d, eye, pm, ones1, acc_sb, dv_sb):
    import os as _os
    xts = big.tile([P, 2, XCOLS], FP8, tag="xts",
                   bufs=int(_os.environ.get("KERNEL_XTS_BUFS", "1")))
    dma_q = _os.environ.get("KERNEL_DMA_Q", "sp")
    for i, (a0, a1) in enumerate(((0, 512), (512, 1536), (1536, 3072),
                                  (3072, 4608), (4608, XCOLS))):
        eng = nc.scalar if (dma_q == "alt" and i % 2 == 1) else nc.sync
        eng.dma_start(out=xts[:, :, a0:a1], in_=xt_d[:, :, a0:a1])

    pe_only = _os.environ.get("KERNEL_PE_ONLY", "0") == "1"
    rs_eng_by_chunk = [_os.environ.get("KERNEL_RS0", "act"),
                       _os.environ.get("KERNEL_RS1", "act"),
                       _os.environ.get("KERNEL_RS_C2", "dve")]
    prev = None
    for r in range(2):
        br = 512 * r
        for c in range(NCH):
            last = (r == 1 and c == NCH - 1)
            # s-matmuls for this chunk (PE), fp8 DoubleRow, K=256 one pass
            pss = []
            for m in range(4):
                ps = pp.tile([P, CW], FP32, tag="super")
                lhsT = xts[:, 0:2, br + 128 * m: br + 128 * m + P]
                for (u0, u1) in _chunk_pieces(m, c):
                    nc.tensor.matmul(ps[:, u0 - CW * c:u1 - CW * c],
                                     lhsT, xts[:, 0:2, br + u0:br + u1],
                                     start=True, stop=True, perf_mode=DR)
                pss.append(ps)
            # column-sum matmuls for the PREVIOUS chunk (they're ready now,
            # and keep PE busy while ACT works on this chunk's exp)
            if prev is not None and not pe_only:
                _colsums(nc, pp, scp, cs_d, ones1, *prev)
            if pe_only:
                prev = None
                continue
            # exp (ACT) -> eo tile [P, 4(m), CW] bf16. Rowsums ride along as
            # ACT accum_out, or go to Pool/DVE reduces (KERNEL_RS01 /
            # KERNEL_RS_C2); the very last chunk always uses ACT accum so
            # the kernel tail stays short.
            rs_eng = {"act": None, "dve": nc.vector,
                      "pool": nc.gpsimd}[rs_eng_by_chunk[c]]
            if last:
                rs_eng = None
            eo = eop.tile([P, 4, CW], BF16, tag="eo")
            for m in range(4):
                lo = max(0, 128 * m - CW * c)
                idx = (r * 4 + m) * 3 + c
                accum = (acc_sb[:, idx:idx + 1] if rs_eng is None else None)
                nc.scalar.activation(
                    out=eo[:, m, lo:CW], in_=pss[m][:, lo:CW],
                    func=mybir.ActivationFunctionType.Exp, scale=EXP_SCALE,
                    accum_out=accum)
            if rs_eng is not None:
                for m in range(4):
                    lo = max(0, 128 * m - CW * c)
                    idx = (r * 4 + m) * 3 + c
                    rs_eng.reduce_sum(acc_sb[:, idx:idx + 1],
                                      eo[:, m, lo:CW],
                                      axis=mybir.AxisListType.X)
            # diag/pair extraction from chunk 0
            if c == 0:
                for m in range(4):
                    gblk = eo[:, m, 128 * m:128 * m + P]
                    tmp = scp.tile([P, P], FP32, tag="gtmp")
                    nc.vector.tensor_tensor(
                        out=tmp, in0=gblk, in1=eye, op=mybir.AluOpType.mult)
                    nc.vector.reduce_sum(
                        dv_sb[:, r * 4 + m:r * 4 + m + 1], tmp,
                        axis=mybir.AxisListType.X)
                    tmp2 = scp.tile([P, P], FP32, tag="gtmp")
                    nc.vector.tensor_tensor(
                        out=tmp2, in0=gblk, in1=pm, op=mybir.AluOpType.mult)
                    nc.vector.reduce_sum(
                        dv_sb[:, 8 + r * 4 + m:8 + r * 4 + m + 1], tmp2,
                        axis=mybir.AxisListType.X)
            prev = (r, c, eo)
    if prev is not None:
        # last chunk: m-interleaved so colsum matmuls overlap the exps
        _colsums(nc, pp, scp, cs_d, ones1, *prev, interleave=True)


def _colsums(nc, pp, scp, cs_d, ones1, r, c, eo, interleave=False):
    """ones-matmul column sums of chunk (r, c)'s exp tile into [1, 512]
    PSUM strips at partition rows 0/32/64 of ONE tile, accumulated over m.
    A single engine copy then moves all three strips to SBUF in parallel
    (per-partition lanes), and one strided-partition DMA ships them to
    cs_d[r, 3c:3c+3, :].

    interleave=True emits m-outer with the last strip's group first so the
    final chunk's stop-matmuls run the moment each exp lands (short tail).
    """
    csp = pp.tile([P, 512], FP32, tag="csp")
    if c == 0:
        # slot order = cs_d slots: diag strip -> row 0 (cols [128:512)),
        # k1 -> row 32, k2 -> row 64
        for mp in range(3):
            l0 = 128 * (mp + 1)
            nc.tensor.matmul(csp[0:1, l0:512], ones1,
                             eo[:, mp, l0:512],
                             start=(mp == 0), stop=(mp == 2))
        ks = [1, 2]
        row = {1: 32, 2: 64}
    else:
        ks = [3 * c, 3 * c + 1, 3 * c + 2]
        row = {k: 32 * i for i, k in enumerate(ks)}
    kso = list(reversed(ks)) if interleave else ks
    order = ([(k, m) for m in range(4) for k in kso] if interleave
             else [(k, m) for k in ks for m in range(4)])
    for k, m in order:
        l0 = 512 * k - CW * c
        nc.tensor.matmul(csp[row[k]:row[k] + 1, 0:512], ones1,
                         eo[:, m, l0:l0 + 512],
                         start=(m == 0), stop=(m == 3),
                         skip_group_check=True)
    scs = scp.tile([P, 512], FP32, tag="scs")
    if os.environ.get("KERNEL_SIM_SAFE", "0") == "1":
        # interp chokes on reading PSUM rows never written; copy row-wise
        # (sim-only mode, slightly overstates DVE time)
        if c == 0:
            nc.vector.memset(scs[0:1, 0:128], 0.0)
        for i in range(3):
            lo = 128 if (c == 0 and i == 0) else 0  # diag strip: [128:512)
            nc.vector.tensor_copy(scs[32 * i:32 * i + 1, lo:],
                                  csp[32 * i:32 * i + 1, lo:])
    elif interleave:
        # last chunk: ACT is idle after the final exp; DVE may still be
        # draining rowsums
        nc.scalar.copy(out=scs[0:65, :], in_=csp[0:65, :])
    else:
        nc.vector.tensor_copy(scs[0:65, :], csp[0:65, :])
    nc.sync.dma_start(out=cs_d[r, 3 * c:3 * c + 3, :], in_=scs[0:65:32, :])


def kernel(x, repeat=None):
    if repeat is None:
        repeat = int(os.environ.get("KERNEL_REPEAT", "1"))
    key = f"nc{repeat}"
    if key not in _CACHE:
        _CACHE[key] = build_nc(repeat)
    nc = _CACHE[key]
    in_maps = _prepare_inputs(x)
    trace = bool(int(os.environ.get("KERNEL_TRACE", "0")))
    res = run_bass_kernel_spmd(nc, in_maps, list(range(NCORES)), trace=trace)
    _CACHE["last_results"] = res
    results = [{"acc": r["accdv"][:, 0:24], "dv": r["accdv"][:, 24:40],
                "cs": r["cs"]} for r in res.results]
    return _combine(results)


if __name__ == "__main__":
    import jax
    cpu = jax.devices("cpu")[0]
    with jax.default_device(cpu):
        import reference
        inputs = reference.setup_inputs()
        expected = float(np.asarray(reference.reference(**inputs)))
    actual = float(mock_kernel(np.asarray(inputs["x"], dtype=np.float32)))
    rel = abs(actual - expected) / abs(expected)
    print(f"mock: expected={expected!r} actual={actual!r} rel={rel:.3e}")


# revision 9
# speedup vs baseline: 1.2606x; 1.2175x over previous
"""Contrastive loss v2: symmetric (upper-triangle) computation on 8 cores.

Math (reference):
    s = cosine similarity matrix of x [8192, 256] (symmetric!)
    d_i = sum_j exp(s_ij/tau) with diag term replaced by 1
    v_i = s[i, i^1];  loss = mean(log d_i - v_i/tau)

v1 computed the FULL s row-block per core (every entry twice globally).
v2 exploits symmetry: each unordered 512-row-group block pair {gi, gj} is
computed ONCE; its exp supplies BOTH row sums (ACT accum / DVE reduce for
rows of gi) and column sums (PE ones-matmul -> [1, W] PSUM strips, DMA'd
out; host scatters to rows of gj).

SPMD via the circulant/roll trick: core c's input columns are rolled by
-c*1024, so one fixed program covers, across cores, all pairs with group
differences delta = 1..8 from both parities. In rolled coords core c
computes two row strips:
    r=0: rows [0:512)    x cols [m*128 : 4608)      (per m-subtile)
    r=1: rows [512:1024) x cols [512 + m*128 : 5120)
Each strip = diag upper-triangle + delta=1..7 once each + delta=8 (which is
covered twice globally; the host drops the duplicate from cores 4-7).

Engines: PE fp8e4+DoubleRow matmuls (K=256 in one 512-wide pass) + bf16
ones-matmul column sums; ACT exp in 3 chunks per (r,m) with fused accum_out
rowsums for chunks 0/1 (+ the last chunk, to shorten the tail); DVE rowsum
reductions for chunk 2, diag/pair mask extraction, and PSUM-strip copies.
Measured 50.7us/core via the repeat-diff bench (78.9us v1 baseline; the
axon terminal drifts ~15% between sessions, same-epoch v1 measured 90.2us).
Keep xts single-buffered and all DMAs on the SP HWDGE queue: early input
DMA firing (bufs=2) and ACT-triggered DMA queues both measured SLOWER on
HW despite simulating faster.
"""

import os
import sys

import numpy as np

sys.path.insert(0, "/opt/trn_rl_repo")

import concourse.bass as bass
import concourse.tile as tile
from concourse import mybir
from concourse.bass_utils import run_bass_kernel_spmd

TAU = 0.1
N = 8192
D = 256
P = 128
NCORES = 8
FP8_SCALE = 16.0
XCOLS = 5120          # rolled cols a core touches: [0, 512 + 4608)
SW = 4608             # strip width in u-coords (relative to 512*r)
CW = 1536             # ACT/PSUM supertile chunk width (3 chunks per strip)
NCH = SW // CW        # 3
FP32 = mybir.dt.float32
FP8 = mybir.dt.float8e4
BF16 = mybir.dt.bfloat16

# rowsum segments per (r, m) = the three ACT/PSUM chunks. The delta=8 dup
# range [4096, 4608) is NOT split out: for cores 4-7 the host subtracts the
# partner core's k=8 column-sum strip instead (bit-identical values).
RS_SEGS = [(0, 1536), (1536, 3072), (3072, 4608)]  # seg0 start is max(128m, 0)

_CACHE = {}


def _quantize(x):
    """normalize rows, scale, quantize to e4m3; returns [256, 8192] fp8."""
    import ml_dtypes
    x = np.ascontiguousarray(np.asarray(x, dtype=np.float32))
    inv = (1.0 / np.sqrt((x * x).sum(axis=1))).astype(np.float32)
    xn = x * inv[:, None]
    return np.ascontiguousarray((xn.T * FP8_SCALE).astype(ml_dtypes.float8_e4m3))


def _prepare_inputs(x):
    xq = _quantize(x)
    eye, pm = _masks()
    in_maps = []
    for c in range(NCORES):
        rolled = np.roll(xq, -c * 2 * 512, axis=1)[:, :XCOLS]
        # [P, 2, XCOLS]: partition-major so one DMA moves both k-halves
        xt = np.ascontiguousarray(
            rolled.reshape(2, P, XCOLS).transpose(1, 0, 2))
        in_maps.append({"xt": xt, "eye": eye, "pm": pm,
                        "ones1": np.ones((P, 1), dtype=_bf16())})
    return in_maps


def _bf16():
    import ml_dtypes
    return ml_dtypes.bfloat16


def _masks():
    mdt = _bf16()
    eye = np.eye(P, dtype=mdt)
    pm = np.zeros((P, P), dtype=mdt)
    idx = np.arange(P)
    pm[idx, idx ^ 1] = mdt(1.0)
    return eye, pm


def mock_core_outputs(x, c):
    """Numpy model of what core c's device program produces (exact fp32
    math, no quantization) — for validating the indexing/combine."""
    x = np.asarray(x, dtype=np.float32)
    xn = x / np.sqrt((x * x).sum(axis=1, keepdims=True))
    xnT = xn.T                                   # [256, 8192]
    rolled = np.roll(xnT, -c * 1024, axis=1)[:, :XCOLS]   # [256, 5120]
    acc = np.zeros((P, 32), dtype=np.float64)
    dv = np.zeros((P, 16), dtype=np.float64)
    cs = np.zeros((2, 9, 512), dtype=np.float64)
    for r in range(2):
        br = 512 * r
        rows = rolled[:, br:br + 512]            # [256, 512] own rows
        # s strip: [512, 4608]
        s = rows.T @ rolled[:, br:br + SW]       # [512 rows, 4608 u-cols]
        e = np.exp(s / TAU)
        for m in range(4):
            em = e[m * 128:(m + 1) * 128]        # [128, 4608]
            for j, (u0, u1) in enumerate(RS_SEGS):
                u0 = max(u0, 128 * m)
                acc[:, (r * 4 + m) * 3 + j] = em[:, u0:u1].sum(axis=1)
            dblk = em[:, 128 * m:128 * (m + 1)]
            dv[:, r * 4 + m] = np.diag(dblk)
            idx = np.arange(P)
            dv[:, 8 + r * 4 + m] = dblk[idx, idx ^ 1]
        # column sums
        for k in range(1, 9):
            u0 = 512 * k
            cs[r, k, :] = e[:, u0:u0 + 512].sum(axis=0)
        # diag-triangle columns [128, 512): only rows strictly above
        for mp in range(3):
            u0, u1 = 128 * (mp + 1), 512
            cs[r, 0, u0:u1] += e[128 * mp:128 * (mp + 1), u0:u1].sum(axis=0)
    return {"acc": acc, "dv": dv, "cs": cs}


def _combine(results):
    d = np.zeros(N, dtype=np.float64)
    diag_exp = np.zeros(N, dtype=np.float64)
    v_exp = np.zeros(N, dtype=np.float64)
    for c in range(NCORES):
        acc = np.asarray(results[c]["acc"], dtype=np.float64)
        dv = np.asarray(results[c]["dv"], dtype=np.float64)
        cs = np.asarray(results[c]["cs"], dtype=np.float64)
        for r in range(2):
            g = (2 * c + r) % 16
            rowbase = g * 512
            for m in range(4):
                rows = rowbase + m * 128 + np.arange(P)
                b = (r * 4 + m) * 3
                d[rows] += acc[:, b:b + 3].sum(axis=1)
                diag_exp[rows] = dv[:, r * 4 + m]
                v_exp[rows] = dv[:, 8 + r * 4 + m]
            if c >= 4:
                # delta=8 block was also computed (transposed) by core c-4;
                # subtract the duplicate via the partner's k=8 colsum strip,
                # whose values are bit-identical to our fused rowsum part
                partner = np.asarray(results[c - 4]["cs"], dtype=np.float64)
                d[rowbase:rowbase + 512] -= partner[r, 8, :]
            for k in range(9):
                if k == 8 and c >= 4:
                    continue                     # delta=8 colsum: add once
                if k == 0:
                    u = np.arange(128, 512)
                    vals = cs[r, 0, 128:512]
                else:
                    u = np.arange(512 * k, 512 * (k + 1))
                    vals = cs[r, k, :]
                cols = (512 * r + u + 1024 * c) % N
                d[cols] += vals
    dfin = d - diag_exp + 1.0
    total = (np.log(dfin) - np.log(v_exp)).sum()
    return np.float32(total / N)


def mock_kernel(x):
    return _combine([mock_core_outputs(x, c) for c in range(NCORES)])


def _split_multi_waits(nc):
    """walrus codegen accepts at most ONE semaphore wait per engine
    instruction; hoist extra waits into standalone InstEventSemaphore ops."""
    n_split = 0
    for blk in nc.m.functions[0].blocks:
        new_insts = []
        for inst in blk.instructions:
            si = inst.sync_info
            tname = type(inst).__name__
            if si is not None and len(si.on_wait) > 1 and tname != "InstEventSemaphore":
                waits = list(si.on_wait)
                for j, w in enumerate(waits[:-1]):
                    es = mybir.InstEventSemaphore(
                        name=f"W-split-{inst.name}-{j}")
                    es.engine = inst.engine
                    es.sync_info = mybir.SyncInfo(on_wait=[w], on_update=[])
                    new_insts.append(es)
                    nc.register_instruction(es)
                    n_split += 1
                inst.sync_info = mybir.SyncInfo(
                    on_wait=[waits[-1]], on_update=list(si.on_update))
            new_insts.append(inst)
        blk.instructions[:] = new_insts
    return n_split


DR = mybir.MatmulPerfMode.DoubleRow
EXP_SCALE = 1.0 / (FP8_SCALE * FP8_SCALE * TAU)


def build_nc(repeat=1):
    import os as _os
    nc = bass.Bass(trn_type="TRN2")
    xt_d = nc.declare_dram_parameter("xt", [P, 2, XCOLS], FP8, isOutput=False)
    eye_d = nc.declare_dram_parameter("eye", [P, P], BF16, isOutput=False)
    pm_d = nc.declare_dram_parameter("pm", [P, P], BF16, isOutput=False)
    ones_d = nc.declare_dram_parameter("ones1", [P, 1], BF16, isOutput=False)
    out_d = nc.declare_dram_parameter("accdv", [P, 40], FP32, isOutput=True)
    cs_d = nc.declare_dram_parameter("cs", [2, 9, 512], FP32, isOutput=True)

    with tile.TileContext(nc) as tc:
        with (
            tc.tile_pool(name="big", bufs=2) as big,
            tc.tile_pool(name="small", bufs=1) as small,
            tc.tile_pool(name="eop", bufs=3) as eop,
            tc.tile_pool(name="sc", bufs=4) as scp,
            tc.tile_pool(name="psum", bufs=2, space="PSUM") as pp,
        ):
            eye = small.tile([P, P], BF16, tag="eye")
            pm = small.tile([P, P], BF16, tag="pm")
            ones1 = small.tile([P, 1], BF16, tag="ones1")
            outsb = small.tile([P, 40], FP32, tag="outsb")
            acc_sb = outsb[:, 0:24]
            dv_sb = outsb[:, 24:40]

            # masks go through the software DGE (gpsimd) so the HWDGE queue
            # is free for the input chunks; warmups run off a memset tile so
            # neither the Exp-table load nor the PE pstate ramp waits on DMA
            nc.gpsimd.dma_start(out=eye, in_=eye_d[:, :])
            nc.gpsimd.dma_start(out=pm, in_=pm_d[:, :])
            nc.gpsimd.dma_start(out=ones1, in_=ones_d[:, :])
            warm_in = small.tile([P, P], BF16, tag="warm_in")
            warm_a = small.tile([P, P], FP32, tag="warm_a")
            warm_s = small.tile([P, 1], FP32, tag="warm_s")
            nc.vector.memset(warm_in, 0.0)
            nc.scalar.activation(out=warm_a, in_=warm_in,
                                 func=mybir.ActivationFunctionType.Exp,
                                 scale=1.0, accum_out=warm_s)
            ps_warm = pp.tile([P, 512], FP32, tag="csp")
            for _w in range(12):
                nc.tensor.matmul(ps_warm[:, 0:P], warm_in, warm_in,
                                 start=True, stop=True)

            import contextlib
            loop_ctx = (tc.For_i(0, repeat, 1)
                        if repeat > 1 else contextlib.nullcontext())
            with loop_ctx:
                _compute_body(nc, tc, pp, eop, scp, big, small,
                              xt_d, cs_d, eye, pm, ones1, acc_sb, dv_sb)

            nc.sync.dma_start(out=out_d[:, :], in_=outsb)
    _split_multi_waits(nc)
    return nc


def _chunk_pieces(m, c):
    """s-matmul piece u-ranges for (m, chunk c): 512-wide, first piece of
    c0 starts at m*128."""
    u0 = CW * c
    if c == 0:
        return [(128 * m, 512), (512, 1024), (1024, 1536)]
    return [(u0, u0 + 512), (u0 + 512, u0 + 1024), (u0 + 1024, u0 + 1536)]


def _compute_body(nc, tc, pp, eop, scp, big, small,
                  xt_d, cs_d, eye, pm, ones1, acc_sb, dv_sb):
    import os as _os
    xts = big.tile([P, 2, XCOLS], FP8, tag="xts",
                   bufs=int(_os.environ.get("KERNEL_XTS_BUFS", "1")))
    dma_q = _os.environ.get("KERNEL_DMA_Q", "sp")
    for i, (a0, a1) in enumerate(((0, 512), (512, 1536), (1536, 3072),
                                  (3072, 4608), (4608, XCOLS))):
        eng = nc.scalar if (dma_q == "alt" and i % 2 == 1) else nc.sync
        eng.dma_start(out=xts[:, :, a0:a1], in_=xt_d[:, :, a0:a1])

    pe_only = _os.environ.get("KERNEL_PE_ONLY", "0") == "1"
    rs_eng_by_chunk = [_os.environ.get("KERNEL_RS0", "act"),
                       _os.environ.get("KERNEL_RS1", "act"),
                       _os.environ.get("KERNEL_RS_C2", "dve")]
    prev = None
    for r in range(2):
        br = 512 * r
        for c in range(NCH):
            last = (r == 1 and c == NCH - 1)
            # s-matmuls for this chunk (PE), fp8 DoubleRow, K=256 one pass
            pss = []
            for m in range(4):
                ps = pp.tile([P, CW], FP32, tag="super")
                lhsT = xts[:, 0:2, br + 128 * m: br + 128 * m + P]
                for (u0, u1) in _chunk_pieces(m, c):
                    nc.tensor.matmul(ps[:, u0 - CW * c:u1 - CW * c],
                                     lhsT, xts[:, 0:2, br + u0:br + u1],
                                     start=True, stop=True, perf_mode=DR)
                pss.append(ps)
            # column-sum matmuls for the PREVIOUS chunk (they're ready now,
            # and keep PE busy while ACT works on this chunk's exp)
            if prev is not None and not pe_only:
                _colsums(nc, pp, scp, cs_d, ones1, *prev)
            if pe_only:
                prev = None
                continue
            # exp (ACT) -> eo tile [P, 4(m), CW] bf16. Rowsums ride along as
            # ACT accum_out, or go to Pool/DVE reduces (KERNEL_RS01 /
            # KERNEL_RS_C2); the very last chunk always uses ACT accum so
            # the kernel tail stays short.
            rs_eng = {"act": None, "dve": nc.vector,
                      "pool": nc.gpsimd}[rs_eng_by_chunk[c]]
            if last:
                rs_eng = None
            eo = eop.tile([P, 4, CW], BF16, tag="eo")
            for m in range(4):
                lo = max(0, 128 * m - CW * c)
                idx = (r * 4 + m) * 3 + c
                accum = (acc_sb[:, idx:idx + 1] if rs_eng is None else None)
                nc.scalar.activation(
                    out=eo[:, m, lo:CW], in_=pss[m][:, lo:CW],
                    func=mybir.ActivationFunctionType.Exp, scale=EXP_SCALE,
                    accum_out=accum)
            if rs_eng is not None:
                for m in range(4):
                    lo = max(0, 128 * m - CW * c)
                    idx = (r * 4 + m) * 3 + c
                    rs_eng.reduce_sum(acc_sb[:, idx:idx + 1],
                                      eo[:, m, lo:CW],
                                      axis=mybir.AxisListType.X)
            # diag/pair extraction from chunk 0
            if c == 0:
                for m in range(4):
                    gblk = eo[:, m, 128 * m:128 * m + P]
                    tmp = scp.tile([P, P], FP32, tag="gtmp")
                    nc.vector.tensor_tensor(
                        out=tmp, in0=gblk, in1=eye, op=mybir.AluOpType.mult)
                    nc.vector.reduce_sum(
                        dv_sb[:, r * 4 + m:r * 4 + m + 1], tmp,
                        axis=mybir.AxisListType.X)
                    tmp2 = scp.tile([P, P], FP32, tag="gtmp")
                    nc.vector.tensor_tensor(
                        out=tmp2, in0=gblk, in1=pm, op=mybir.AluOpType.mult)
                    nc.vector.reduce_sum(
                        dv_sb[:, 8 + r * 4 + m:8 + r * 4 + m + 1], tmp2,
                        axis=mybir.AxisListType.X)
            prev = (r, c, eo)
    if prev is not None:
        # last chunk: m-interleaved so colsum matmuls overlap the exps
        _colsums(nc, pp, scp, cs_d, ones1, *prev, interleave=True)


def _colsums(nc, pp, scp, cs_d, ones1, r, c, eo, interleave=False):
    """ones-matmul column sums of chunk (r, c)'s exp tile into [1, 512]
    PSUM strips at partition rows 0/32/64 of ONE tile, accumulated over m.
    A single engine copy then moves all three strips to SBUF in parallel
    (per-partition lanes), and one strided-partition DMA ships them to
    cs_d[r, 3c:3c+3, :].

    interleave=True emits m-outer with the last strip's group first so the
    final chunk's stop-matmuls run the moment each exp lands (short tail).
    """
    csp = pp.tile([P, 512], FP32, tag="csp")
    if c == 0:
        # slot order = cs_d slots: diag strip -> row 0 (cols [128:512)),
        # k1 -> row 32, k2 -> row 64
        for mp in range(3):
            l0 = 128 * (mp + 1)
            nc.tensor.matmul(csp[0:1, l0:512], ones1,
                             eo[:, mp, l0:512],
                             start=(mp == 0), stop=(mp == 2))
        ks = [1, 2]
        row = {1: 32, 2: 64}
    else:
        ks = [3 * c, 3 * c + 1, 3 * c + 2]
        row = {k: 32 * i for i, k in enumerate(ks)}
    kso = list(reversed(ks)) if interleave else ks
    order = ([(k, m) for m in range(4) for k in kso] if interleave
             else [(k, m) for k in ks for m in range(4)])
    for k, m in order:
        l0 = 512 * k - CW * c
        nc.tensor.matmul(csp[row[k]:row[k] + 1, 0:512], ones1,
                         eo[:, m, l0:l0 + 512],
                         start=(m == 0), stop=(m == 3),
                         skip_group_check=True)
    scs = scp.tile([P, 512], FP32, tag="scs")
    if os.environ.get("KERNEL_SIM_SAFE", "0") == "1":
        # interp chokes on reading PSUM rows never written; copy row-wise
        # (sim-only mode, slightly overstates DVE time)
        if c == 0:
            nc.vector.memset(scs[0:1, 0:128], 0.0)
        for i in range(3):
            lo = 128 if (c == 0 and i == 0) else 0  # diag strip: [128:512)
            nc.vector.tensor_copy(scs[32 * i:32 * i + 1, lo:],
                                  csp[32 * i:32 * i + 1, lo:])
    elif interleave:
        # last chunk: ACT is idle after the final exp; DVE may still be
        # draining rowsums
        nc.scalar.copy(out=scs[0:65, :], in_=csp[0:65, :])
    else:
        nc.vector.tensor_copy(scs[0:65, :], csp[0:65, :])
    nc.sync.dma_start(out=cs_d[r, 3 * c:3 * c + 3, :], in_=scs[0:65:32, :])


def kernel(x, repeat=None):
    if repeat is None:
        repeat = int(os.environ.get("KERNEL_REPEAT", "1"))
    key = f"nc{repeat}"
    if key not in _CACHE:
        _CACHE[key] = build_nc(repeat)
    nc = _CACHE[key]
    in_maps = _prepare_inputs(x)
    trace = bool(int(os.environ.get("KERNEL_TRACE", "0")))
    res = run_bass_kernel_spmd(nc, in_maps, list(range(NCORES)), trace=trace)
    _CACHE["last_results"] = res
    results = [{"acc": r["accdv"][:, 0:24], "dv": r["accdv"][:, 24:40],
                "cs": r["cs"]} for r in res.results]
    return _combine(results)


if __name__ == "__main__":
    import jax
    cpu = jax.devices("cpu")[0]
    with jax.default_device(cpu):
        import reference
        inputs = reference.setup_inputs()
        expected = float(np.asarray(reference.reference(**inputs)))
    actual = float(mock_kernel(np.asarray(inputs["x"], dtype=np.float32)))
    rel = abs(actual - expected) / abs(expected)
    print(f"mock: expected={expected!r} actual={actual!r} rel={rel:.3e}")


# revision 10
# speedup vs baseline: 1.3099x; 1.0390x over previous
"""Contrastive loss v2: symmetric (upper-triangle) computation on 8 cores.

Math (reference):
    s = cosine similarity matrix of x [8192, 256] (symmetric!)
    d_i = sum_j exp(s_ij/tau) with diag term replaced by 1
    v_i = s[i, i^1];  loss = mean(log d_i - v_i/tau)

v1 computed the FULL s row-block per core (every entry twice globally).
v2 exploits symmetry: each unordered 512-row-group block pair {gi, gj} is
computed ONCE; its exp supplies BOTH row sums (ACT accum / DVE reduce for
rows of gi) and column sums (PE ones-matmul -> [1, W] PSUM strips, DMA'd
out; host scatters to rows of gj).

SPMD via the circulant/roll trick: core c's input columns are rolled by
-c*1024, so one fixed program covers, across cores, all pairs with group
differences delta = 1..8 from both parities. In rolled coords core c
computes two row strips:
    r=0: rows [0:512)    x cols [m*128 : 4608)      (per m-subtile)
    r=1: rows [512:1024) x cols [512 + m*128 : 5120)
Each strip = diag upper-triangle + delta=1..7 once each + delta=8 (which is
covered twice globally; the host drops the duplicate from cores 4-7).

Engines: PE fp8e4+DoubleRow matmuls (K=256 in one 512-wide pass) + bf16
ones-matmul column sums; ACT exp (no accum -> 3 big chunks per (r,m));
DVE row-sum reductions (bf16 2x) + diag/pair mask extraction.
"""

import os
import sys

import numpy as np

sys.path.insert(0, "/opt/trn_rl_repo")

import concourse.bass as bass
import concourse.tile as tile
from concourse import mybir
from concourse.bass_utils import run_bass_kernel_spmd

TAU = 0.1
N = 8192
D = 256
P = 128
NCORES = 8
FP8_SCALE = 16.0
XCOLS = 5120          # rolled cols a core touches: [0, 512 + 4608)
SW = 4608             # strip width in u-coords (relative to 512*r)
CW = 1536             # ACT/PSUM supertile chunk width (3 chunks per strip)
NCH = SW // CW        # 3
FP32 = mybir.dt.float32
FP8 = mybir.dt.float8e4
BF16 = mybir.dt.bfloat16

# rowsum segments per (r, m) = the three ACT/PSUM chunks. The delta=8 dup
# range [4096, 4608) is NOT split out: for cores 4-7 the host subtracts the
# partner core's k=8 column-sum strip instead (bit-identical values).
RS_SEGS = [(0, 1536), (1536, 3072), (3072, 4608)]  # seg0 start is max(128m, 0)

_CACHE = {}


def _quantize(x):
    """normalize rows, scale, quantize to e4m3; returns [256, 8192] fp8."""
    import ml_dtypes
    x = np.ascontiguousarray(np.asarray(x, dtype=np.float32))
    inv = (1.0 / np.sqrt((x * x).sum(axis=1))).astype(np.float32)
    xn = x * inv[:, None]
    return np.ascontiguousarray((xn.T * FP8_SCALE).astype(ml_dtypes.float8_e4m3))


def _prepare_inputs(x):
    xq = _quantize(x)
    eye, pm = _masks()
    in_maps = []
    for c in range(NCORES):
        rolled = np.roll(xq, -c * 2 * 512, axis=1)[:, :XCOLS]
        # [P, 2, XCOLS]: partition-major so one DMA moves both k-halves
        xt = np.ascontiguousarray(
            rolled.reshape(2, P, XCOLS).transpose(1, 0, 2))
        in_maps.append({"xt": xt, "eye": eye, "pm": pm,
                        "ones1": np.ones((P, 1), dtype=_bf16())})
    return in_maps


def _bf16():
    import ml_dtypes
    return ml_dtypes.bfloat16


def _masks():
    mdt = _bf16()
    eye = np.eye(P, dtype=mdt)
    pm = np.zeros((P, P), dtype=mdt)
    idx = np.arange(P)
    pm[idx, idx ^ 1] = mdt(1.0)
    return eye, pm


def mock_core_outputs(x, c):
    """Numpy model of what core c's device program produces (exact fp32
    math, no quantization) — for validating the indexing/combine."""
    x = np.asarray(x, dtype=np.float32)
    xn = x / np.sqrt((x * x).sum(axis=1, keepdims=True))
    xnT = xn.T                                   # [256, 8192]
    rolled = np.roll(xnT, -c * 1024, axis=1)[:, :XCOLS]   # [256, 5120]
    acc = np.zeros((P, 32), dtype=np.float64)
    dv = np.zeros((P, 16), dtype=np.float64)
    cs = np.zeros((2, 9, 512), dtype=np.float64)
    for r in range(2):
        br = 512 * r
        rows = rolled[:, br:br + 512]            # [256, 512] own rows
        # s strip: [512, 4608]
        s = rows.T @ rolled[:, br:br + SW]       # [512 rows, 4608 u-cols]
        e = np.exp(s / TAU)
        for m in range(4):
            em = e[m * 128:(m + 1) * 128]        # [128, 4608]
            for j, (u0, u1) in enumerate(RS_SEGS):
                u0 = max(u0, 128 * m)
                acc[:, (r * 4 + m) * 3 + j] = em[:, u0:u1].sum(axis=1)
            dblk = em[:, 128 * m:128 * (m + 1)]
            dv[:, r * 4 + m] = np.diag(dblk)
            idx = np.arange(P)
            dv[:, 8 + r * 4 + m] = dblk[idx, idx ^ 1]
        # column sums
        for k in range(1, 9):
            u0 = 512 * k
            cs[r, k, :] = e[:, u0:u0 + 512].sum(axis=0)
        # diag-triangle columns [128, 512): only rows strictly above
        for mp in range(3):
            u0, u1 = 128 * (mp + 1), 512
            cs[r, 0, u0:u1] += e[128 * mp:128 * (mp + 1), u0:u1].sum(axis=0)
    return {"acc": acc, "dv": dv, "cs": cs}


def _combine(results):
    d = np.zeros(N, dtype=np.float64)
    diag_exp = np.zeros(N, dtype=np.float64)
    v_exp = np.zeros(N, dtype=np.float64)
    for c in range(NCORES):
        acc = np.asarray(results[c]["acc"], dtype=np.float64)
        dv = np.asarray(results[c]["dv"], dtype=np.float64)
        cs = np.asarray(results[c]["cs"], dtype=np.float64)
        for r in range(2):
            g = (2 * c + r) % 16
            rowbase = g * 512
            for m in range(4):
                rows = rowbase + m * 128 + np.arange(P)
                b = (r * 4 + m) * 3
                d[rows] += acc[:, b:b + 3].sum(axis=1)
                diag_exp[rows] = dv[:, r * 4 + m]
                v_exp[rows] = dv[:, 8 + r * 4 + m]
            if c >= 4:
                # delta=8 block was also computed (transposed) by core c-4;
                # subtract the duplicate via the partner's k=8 colsum strip,
                # whose values are bit-identical to our fused rowsum part
                partner = np.asarray(results[c - 4]["cs"], dtype=np.float64)
                d[rowbase:rowbase + 512] -= partner[r, 8, :]
            for k in range(9):
                if k == 8 and c >= 4:
                    continue                     # delta=8 colsum: add once
                if k == 0:
                    u = np.arange(128, 512)
                    vals = cs[r, 0, 128:512]
                else:
                    u = np.arange(512 * k, 512 * (k + 1))
                    vals = cs[r, k, :]
                cols = (512 * r + u + 1024 * c) % N
                d[cols] += vals
    dfin = d - diag_exp + 1.0
    total = (np.log(dfin) - np.log(v_exp)).sum()
    return np.float32(total / N)


def mock_kernel(x):
    return _combine([mock_core_outputs(x, c) for c in range(NCORES)])


def _split_multi_waits(nc):
    """walrus codegen accepts at most ONE semaphore wait per engine
    instruction; hoist extra waits into standalone InstEventSemaphore ops."""
    n_split = 0
    for blk in nc.m.functions[0].blocks:
        new_insts = []
        for inst in blk.instructions:
            si = inst.sync_info
            tname = type(inst).__name__
            if si is not None and len(si.on_wait) > 1 and tname != "InstEventSemaphore":
                waits = list(si.on_wait)
                for j, w in enumerate(waits[:-1]):
                    es = mybir.InstEventSemaphore(
                        name=f"W-split-{inst.name}-{j}")
                    es.engine = inst.engine
                    es.sync_info = mybir.SyncInfo(on_wait=[w], on_update=[])
                    new_insts.append(es)
                    nc.register_instruction(es)
                    n_split += 1
                inst.sync_info = mybir.SyncInfo(
                    on_wait=[waits[-1]], on_update=list(si.on_update))
            new_insts.append(inst)
        blk.instructions[:] = new_insts
    return n_split


DR = mybir.MatmulPerfMode.DoubleRow
EXP_SCALE = 1.0 / (FP8_SCALE * FP8_SCALE * TAU)


def build_nc(repeat=1):
    import os as _os
    nc = bass.Bass(trn_type="TRN2")
    xt_d = nc.declare_dram_parameter("xt", [P, 2, XCOLS], FP8, isOutput=False)
    eye_d = nc.declare_dram_parameter("eye", [P, P], BF16, isOutput=False)
    pm_d = nc.declare_dram_parameter("pm", [P, P], BF16, isOutput=False)
    ones_d = nc.declare_dram_parameter("ones1", [P, 1], BF16, isOutput=False)
    out_d = nc.declare_dram_parameter("accdv", [P, 40], FP32, isOutput=True)
    cs_d = nc.declare_dram_parameter("cs", [2, 9, 512], FP32, isOutput=True)

    with tile.TileContext(nc) as tc:
        with (
            tc.tile_pool(name="big", bufs=2) as big,
            tc.tile_pool(name="small", bufs=1) as small,
            tc.tile_pool(name="eop", bufs=3) as eop,
            tc.tile_pool(name="sc", bufs=4) as scp,
            tc.tile_pool(name="psum", bufs=2, space="PSUM") as pp,
        ):
            eye = small.tile([P, P], BF16, tag="eye")
            pm = small.tile([P, P], BF16, tag="pm")
            ones1 = small.tile([P, 1], BF16, tag="ones1")
            outsb = small.tile([P, 40], FP32, tag="outsb")
            acc_sb = outsb[:, 0:24]
            dv_sb = outsb[:, 24:40]

            # masks go through the software DGE (gpsimd) so the HWDGE queue
            # is free for the input chunks; warmups run off a memset tile so
            # neither the Exp-table load nor the PE pstate ramp waits on DMA
            nc.gpsimd.dma_start(out=eye, in_=eye_d[:, :])
            nc.gpsimd.dma_start(out=pm, in_=pm_d[:, :])
            nc.gpsimd.dma_start(out=ones1, in_=ones_d[:, :])
            warm_in = small.tile([P, P], BF16, tag="warm_in")
            warm_a = small.tile([P, P], FP32, tag="warm_a")
            warm_s = small.tile([P, 1], FP32, tag="warm_s")
            nc.vector.memset(warm_in, 0.0)
            nc.scalar.activation(out=warm_a, in_=warm_in,
                                 func=mybir.ActivationFunctionType.Exp,
                                 scale=1.0, accum_out=warm_s)
            ps_warm = pp.tile([P, 512], FP32, tag="csp")
            for _w in range(12):
                nc.tensor.matmul(ps_warm[:, 0:P], warm_in, warm_in,
                                 start=True, stop=True)
            # head prefetch buffer: cols [0:2048] (both r-strips' weights +
            # chunk c0), reloaded late each iteration for the next one so
            # the loop boundary never waits on input DMA
            xh = big.tile([P, 2, 2048], FP8, tag="xh")
            nc.sync.dma_start(out=xh[:, :, :], in_=xt_d[:, :, 0:2048])

            import contextlib
            loop_ctx = (tc.For_i(0, repeat, 1)
                        if repeat > 1 else contextlib.nullcontext())
            with loop_ctx:
                _compute_body(nc, tc, pp, eop, scp, big, small,
                              xt_d, cs_d, eye, pm, ones1, acc_sb, dv_sb,
                              xh, reload_head=repeat > 1)

            nc.sync.dma_start(out=out_d[:, :], in_=outsb)
    _split_multi_waits(nc)
    return nc


def _chunk_pieces(m, c):
    """s-matmul piece u-ranges for (m, chunk c): 512-wide, first piece of
    c0 starts at m*128."""
    u0 = CW * c
    if c == 0:
        return [(128 * m, 512), (512, 1024), (1024, 1536)]
    return [(u0, u0 + 512), (u0 + 512, u0 + 1024), (u0 + 1024, u0 + 1536)]


def _compute_body(nc, tc, pp, eop, scp, big, small,
                  xt_d, cs_d, eye, pm, ones1, acc_sb, dv_sb,
                  xh, reload_head=False):
    import os as _os
    xts = big.tile([P, 2, XCOLS], FP8, tag="xts",
                   bufs=int(_os.environ.get("KERNEL_XTS_BUFS", "1")))
    for (a0, a1) in ((2048, 3072), (3072, 4608), (4608, XCOLS)):
        nc.sync.dma_start(out=xts[:, :, a0:a1], in_=xt_d[:, :, a0:a1])

    def xsl(a0, a1):
        return xh[:, 0:2, a0:a1] if a1 <= 2048 else xts[:, 0:2, a0:a1]

    pe_only = _os.environ.get("KERNEL_PE_ONLY", "0") == "1"
    rs_eng_by_chunk = [_os.environ.get("KERNEL_RS0", "act"),
                       _os.environ.get("KERNEL_RS1", "act"),
                       _os.environ.get("KERNEL_RS_C2", "dve")]
    prev = None
    for r in range(2):
        br = 512 * r
        for c in range(NCH):
            last = (r == 1 and c == NCH - 1)
            # s-matmuls for this chunk (PE), fp8 DoubleRow, K=256 one pass
            pss = []
            for m in range(4):
                ps = pp.tile([P, CW], FP32, tag="super")
                lhsT = xsl(br + 128 * m, br + 128 * m + P)
                for (u0, u1) in _chunk_pieces(m, c):
                    nc.tensor.matmul(ps[:, u0 - CW * c:u1 - CW * c],
                                     lhsT, xsl(br + u0, br + u1),
                                     start=True, stop=True, perf_mode=DR)
                pss.append(ps)
            if last and reload_head:
                # all head readers are done after the final chunk's
                # s-matmuls: refill it for the next iteration now
                nc.sync.dma_start(out=xh[:, :, :], in_=xt_d[:, :, 0:2048])
            # column-sum matmuls for the PREVIOUS chunk (they're ready now,
            # and keep PE busy while ACT works on this chunk's exp)
            if prev is not None and not pe_only:
                _colsums(nc, pp, scp, cs_d, ones1, *prev)
            if pe_only:
                prev = None
                continue
            # exp (ACT) -> eo tile [P, 4(m), CW] bf16. Rowsums ride along as
            # ACT accum_out, or go to Pool/DVE reduces (KERNEL_RS01 /
            # KERNEL_RS_C2); the very last chunk always uses ACT accum so
            # the kernel tail stays short.
            rs_eng = {"act": None, "dve": nc.vector,
                      "pool": nc.gpsimd}[rs_eng_by_chunk[c]]
            if last:
                rs_eng = None
            eo = eop.tile([P, 4, CW], BF16, tag="eo")
            for m in range(4):
                lo = max(0, 128 * m - CW * c)
                idx = (r * 4 + m) * 3 + c
                accum = (acc_sb[:, idx:idx + 1] if rs_eng is None else None)
                nc.scalar.activation(
                    out=eo[:, m, lo:CW], in_=pss[m][:, lo:CW],
                    func=mybir.ActivationFunctionType.Exp, scale=EXP_SCALE,
                    accum_out=accum)
            if rs_eng is not None:
                for m in range(4):
                    lo = max(0, 128 * m - CW * c)
                    idx = (r * 4 + m) * 3 + c
                    rs_eng.reduce_sum(acc_sb[:, idx:idx + 1],
                                      eo[:, m, lo:CW],
                                      axis=mybir.AxisListType.X)
            # diag/pair extraction from chunk 0
            if c == 0:
                for m in range(4):
                    gblk = eo[:, m, 128 * m:128 * m + P]
                    tmp = scp.tile([P, P], FP32, tag="gtmp")
                    nc.vector.tensor_tensor(
                        out=tmp, in0=gblk, in1=eye, op=mybir.AluOpType.mult)
                    nc.vector.reduce_sum(
                        dv_sb[:, r * 4 + m:r * 4 + m + 1], tmp,
                        axis=mybir.AxisListType.X)
                    tmp2 = scp.tile([P, P], FP32, tag="gtmp")
                    nc.vector.tensor_tensor(
                        out=tmp2, in0=gblk, in1=pm, op=mybir.AluOpType.mult)
                    nc.vector.reduce_sum(
                        dv_sb[:, 8 + r * 4 + m:8 + r * 4 + m + 1], tmp2,
                        axis=mybir.AxisListType.X)
            prev = (r, c, eo)
    if prev is not None:
        # last chunk: m-interleaved so colsum matmuls overlap the exps
        _colsums(nc, pp, scp, cs_d, ones1, *prev, interleave=True)


def _colsums(nc, pp, scp, cs_d, ones1, r, c, eo, interleave=False):
    """ones-matmul column sums of chunk (r, c)'s exp tile into [1, 512]
    PSUM strips at partition rows 0/32/64 of ONE tile, accumulated over m.
    A single engine copy then moves all three strips to SBUF in parallel
    (per-partition lanes), and one strided-partition DMA ships them to
    cs_d[r, 3c:3c+3, :].

    interleave=True emits m-outer with the last strip's group first so the
    final chunk's stop-matmuls run the moment each exp lands (short tail).
    """
    csp = pp.tile([P, 512], FP32, tag="csp")
    if c == 0:
        # slot order = cs_d slots: diag strip -> row 0 (cols [128:512)),
        # k1 -> row 32, k2 -> row 64
        for mp in range(3):
            l0 = 128 * (mp + 1)
            nc.tensor.matmul(csp[0:1, l0:512], ones1,
                             eo[:, mp, l0:512],
                             start=(mp == 0), stop=(mp == 2))
        ks = [1, 2]
        row = {1: 32, 2: 64}
    else:
        ks = [3 * c, 3 * c + 1, 3 * c + 2]
        row = {k: 32 * i for i, k in enumerate(ks)}
    kso = list(reversed(ks)) if interleave else ks
    order = ([(k, m) for m in range(4) for k in kso] if interleave
             else [(k, m) for k in ks for m in range(4)])
    for k, m in order:
        l0 = 512 * k - CW * c
        nc.tensor.matmul(csp[row[k]:row[k] + 1, 0:512], ones1,
                         eo[:, m, l0:l0 + 512],
                         start=(m == 0), stop=(m == 3),
                         skip_group_check=True)
    scs = scp.tile([P, 512], FP32, tag="scs")
    if os.environ.get("KERNEL_SIM_SAFE", "0") == "1":
        # interp chokes on reading PSUM rows never written; copy row-wise
        # (sim-only mode, slightly overstates DVE time)
        if c == 0:
            nc.vector.memset(scs[0:1, 0:128], 0.0)
        for i in range(3):
            lo = 128 if (c == 0 and i == 0) else 0  # diag strip: [128:512)
            nc.vector.tensor_copy(scs[32 * i:32 * i + 1, lo:],
                                  csp[32 * i:32 * i + 1, lo:])
    elif interleave:
        # last chunk: ACT is idle after the final exp; DVE may still be
        # draining rowsums
        nc.scalar.copy(out=scs[0:65, :], in_=csp[0:65, :])
    else:
        nc.vector.tensor_copy(scs[0:65, :], csp[0:65, :])
    nc.sync.dma_start(out=cs_d[r, 3 * c:3 * c + 3, :], in_=scs[0:65:32, :])


def kernel(x, repeat=None):
    if repeat is None:
        repeat = int(os.environ.get("KERNEL_REPEAT", "1"))
    key = f"nc{repeat}"
    if key not in _CACHE:
        _CACHE[key] = build_nc(repeat)
    nc = _CACHE[key]
    in_maps = _prepare_inputs(x)
    trace = bool(int(os.environ.get("KERNEL_TRACE", "0")))
    res = run_bass_kernel_spmd(nc, in_maps, list(range(NCORES)), trace=trace)
    _CACHE["last_results"] = res
    results = [{"acc": r["accdv"][:, 0:24], "dv": r["accdv"][:, 24:40],
                "cs": r["cs"]} for r in res.results]
    return _combine(results)


if __name__ == "__main__":
    import jax
    cpu = jax.devices("cpu")[0]
    with jax.default_device(cpu):
        import reference
        inputs = reference.setup_inputs()
        expected = float(np.asarray(reference.reference(**inputs)))
    actual = float(mock_kernel(np.asarray(inputs["x"], dtype=np.float32)))
    rel = abs(actual - expected) / abs(expected)
    print(f"mock: expected={expected!r} actual={actual!r} rel={rel:.3e}")
